# revision 17
# baseline (speedup 1.0000x reference)
"""Trainium2 Bass kernel for nn_CNNQNetwork (dueling CNN Q-network).

Sharding: pure data parallel — batch 4096 split as 512 samples on each of the
8 NeuronCores; all weights replicated.

v2 design (vs baseline): fewer, larger ops everywhere; GroupNorm stats
restructured so the PE stays dense and the DVE does two big reduces per
(block, 128-sample chunk) instead of many small strided ones.

Per-core layout: activations in SBUF as [channel(partition), spatial, batch].
Per (block, q=128-sample chunk): conv taps accumulate into a PSUM tile
[128, S, 128] (3 banks); matmuls are split at 2KB PSUM bank boundaries
(rectangular (i,j) sub-windows). GroupNorm(1 group):
  - zs[c,b] = sum_s z via DVE reduce (PSUM f32)
  - sq = z^2 via ScalarE -> [c,b,s] bf16, sqs[c,b] via dense DVE reduce
  - cross-channel sums via ones-matmul -> [1,512] rows in PSUM
  - row math: mu (ScalarE copy*1/CS), mu2 (ScalarE square*1/CS),
    ve (DVE STT), sd=sqrt(ve+eps) (ScalarE), r=1/sd (DVE recip_approx_fast)
  - mean subtraction folded into the conv PSUM as a K=1 matmul (lhsT=-1,
    rhs=mu row broadcast)
  - u = relu(z - mu) (ScalarE evict), feat = u * (r bcast) (GpSimd)
Per-channel gammas are folded into consuming conv weights and the head W1
host-side; per-sample 1/std appears only via feat (scale invariance of
GroupNorm makes intermediate u unnormalized-safe). The dueling-head algebra
is folded into the second linear layer host-side.
"""

import numpy as np
import ml_dtypes

BF16 = ml_dtypes.bfloat16
B_TOTAL = 4096
NCORES = 8
BC = B_TOTAL // NCORES  # 512 samples per core
D = 128
EPS = 1e-5

# blocks: (name, src, kind, Hi, Wi, Ho, Wo)   kind 'h' = (1,2) kernel, 'v' = (2,1)
BLOCKS = [
    ("h1", "x", "h", 4, 4, 4, 3),
    ("v1", "x", "v", 4, 4, 3, 4),
    ("hh", "h1", "h", 4, 3, 4, 2),
    ("hv", "h1", "v", 4, 3, 3, 3),
    ("vh", "v1", "h", 3, 4, 3, 3),
    ("vv", "v1", "v", 3, 4, 2, 4),
]
S_OF = {n: ho * wo for (n, _, _, _, _, ho, wo) in BLOCKS}
NK = sum(S_OF.values())  # 58 K-slices of 128 for the head matmul

_cache = {}
DEBUG_DUMP = False


def _regions(Ho, Wo):
    """Rectangular (i0, ni, j0, nj) output sub-windows whose flattened
    (s, b128) PSUM columns stay within one 2KB bank (4 s-positions)."""
    regs = []
    if Wo == 4:
        for i in range(Ho):
            regs.append((i, 1, 0, 4))
    elif Wo == 2:
        for i in range(0, Ho, 2):
            regs.append((i, 2, 0, 2))
    else:  # Wo == 3
        for i in range(Ho):
            j = 0
            while j < Wo:
                s = i * Wo + j
                jn = min(Wo - j, 4 - (s % 4))
                regs.append((i, 1, j, jn))
                j += jn
    return regs


def _build():
    """Build the Bass program once. Returns nc."""
    import concourse.bass as bass
    import concourse.tile as tile
    import concourse.mybir as mybir
    from concourse import bacc
    from concourse.masks import make_identity
    from contextlib import ExitStack

    dt = mybir.dt
    Alu = mybir.AluOpType
    Act = mybir.ActivationFunctionType

    nc = bacc.Bacc(
        "TRN2",
        target_bir_lowering=False,
        debug=False,
        enable_asserts=False,
        num_devices=NCORES,
    )

    # ---- DRAM I/O ----
    xt_d = nc.dram_tensor("xt", [64, 16, BC], dt.bfloat16, kind="ExternalInput")
    cw1_d = nc.dram_tensor("cw1", [64, 128], dt.bfloat16, kind="ExternalInput")
    cw_d = nc.dram_tensor("cw", [128, 8 * 128], dt.bfloat16, kind="ExternalInput")
    hw_d = nc.dram_tensor("hw", [4, 128, NK * 128], dt.bfloat16, kind="ExternalInput")
    fw_d = nc.dram_tensor("fw", [128, 16], dt.bfloat16, kind="ExternalInput")
    hb_d = nc.dram_tensor("hb", [128, 4], dt.float32, kind="ExternalInput")
    b2_d = nc.dram_tensor("b2", [4, 1], dt.float32, kind="ExternalInput")
    out_d = nc.dram_tensor("out", [BC, 4], dt.float32, kind="ExternalOutput")
    if DEBUG_DUMP:
        dbg_u = {
            n: nc.dram_tensor(f"dbg_u_{n}", [128, 12, BC], dt.bfloat16, kind="ExternalOutput")
            for n in ("h1", "v1")
        }
        dbg_f = {
            n: nc.dram_tensor(f"dbg_f_{n}", [128, S_OF[n], BC], dt.bfloat16, kind="ExternalOutput")
            for n in S_OF
        }

    with tile.TileContext(nc) as tc, ExitStack() as ctx:
        singles = ctx.enter_context(tc.tile_pool(name="singles", bufs=1))
        rows = ctx.enter_context(tc.tile_pool(name="rows", bufs=2))
        sqp = ctx.enter_context(tc.tile_pool(name="sqp", bufs=3))
        statp = ctx.enter_context(tc.tile_pool(name="statp", bufs=2))
        uleafp = ctx.enter_context(tc.tile_pool(name="uleafp", bufs=2))
        gsbp = ctx.enter_context(tc.tile_pool(name="gsbp", bufs=2))
        hwp = ctx.enter_context(tc.tile_pool(name="hwp", bufs=2))
        hidp = ctx.enter_context(tc.tile_pool(name="hidp", bufs=1))
        hp0 = ctx.enter_context(tc.tile_pool(name="hp0", bufs=1, space="PSUM"))

        # persistent SBUF tensors
        fw_sb = singles.tile([128, 16], dt.bfloat16, tag="fw", name="fw")
        hb_sb = singles.tile([128, 4], dt.float32, tag="hb", name="hb")
        b2_sb = singles.tile([4, 1], dt.float32, tag="b2", name="b2")
        ident = singles.tile([128, 128], dt.float32, tag="ident", name="ident")
        ones_col = singles.tile([128, 1], dt.bfloat16, tag="ones_col", name="ones_col")
        posones = singles.tile([1, 128], dt.bfloat16, tag="posones", name="posones")
        negones = singles.tile([1, 128], dt.bfloat16, tag="negones", name="negones")
        eps1 = singles.tile([1, 1], dt.float32, tag="eps1", name="eps1")
        nc.vector.memset(eps1[:], EPS)
        nc.vector.memset(ones_col[:], 1.0)
        nc.vector.memset(posones[:], 1.0)
        nc.vector.memset(negones[:], -1.0)

        nc.sync.dma_start(fw_sb[:], fw_d[:])
        nc.sync.dma_start(hb_sb[:], hb_d[:])
        nc.sync.dma_start(b2_sb[:], b2_d[:])
        make_identity(nc, ident[:])

        # head weights for mtiles 0/1 stream in during the conv phase
        hws01 = []
        for mt in range(2):
            hws = hwp.tile([128, NK * 128], dt.bfloat16, tag="hws", name=f"hws{mt}")
            nc.sync.dma_start(hws[:], hw_d[mt])
            hws01.append(hws)
        # mtile-0 head accumulator lives through the conv phase
        psH0 = hp0.tile([128, BC], dt.float32, tag="psH0", name="psH0")

        # activations (bf16, [c, s, b])
        feat = {}
        for name, _, _, _, _, ho, wo in BLOCKS:
            feat[name] = singles.tile(
                [128, ho * wo, BC], dt.bfloat16, tag=f"f_{name}", name=f"f_{name}"
            )
        u_keep = {
            "h1": singles.tile([128, 12, BC], dt.bfloat16, tag="u_h1", name="u_h1"),
            "v1": singles.tile([128, 12, BC], dt.bfloat16, tag="u_v1", name="u_v1"),
        }

        with (
            tc.tile_pool(name="convp", bufs=1) as convp,
            tc.tile_pool(name="zp", bufs=2, space="PSUM") as zp,
            tc.tile_pool(name="gp", bufs=1, space="PSUM") as gp,
        ):
            xt_sb = convp.tile([64, 16, BC], dt.bfloat16, tag="xt", name="xt")
            cw1_sb = convp.tile([64, 128], dt.bfloat16, tag="cw1", name="cw1")
            cw_sb = convp.tile([128, 8 * 128], dt.bfloat16, tag="cw", name="cw")
            nc.sync.dma_start(cw1_sb[:], cw1_d[:])
            nc.sync.dma_start(cw_sb[:], cw_d[:])
            for q in range(4):
                b0 = q * 128
                nc.sync.dma_start(
                    xt_sb[:, :, b0 : b0 + 128], xt_d[:, :, b0 : b0 + 128]
                )

            for bi, (name, src, kind, Hi, Wi, Ho, Wo) in enumerate(BLOCKS):
                S = Ho * Wo
                CS = 128 * S
                first = src == "x"
                regs = _regions(Ho, Wo)

                if first:
                    base = 0 if kind == "h" else 32
                    sview = xt_sb[base : base + 32].rearrange(
                        "c (i j) b -> c i j b", i=Hi
                    )
                else:
                    sview = u_keep[src][:].rearrange("c (i j) b -> c i j b", i=Hi)

                def rhs_win(t, i0, ni, j0, nj, b0):
                    if kind == "h":
                        return sview[:, i0 : i0 + ni, j0 + t : j0 + t + nj, b0 : b0 + 128]
                    else:
                        return sview[:, i0 + t : i0 + t + ni, j0 : j0 + nj, b0 : b0 + 128]

                u_dst = u_keep[name] if name in u_keep else uleafp.tile(
                    [128, 9, BC], dt.bfloat16, tag="uleaf", name=f"u_{name}"
                )

                zs = statp.tile([128, BC], dt.bfloat16, tag="zs", name="zs")
                sqs = statp.tile([128, BC], dt.bfloat16, tag="sqs", name="sqs")

                # One PSUM bank serves both the stats rows (zs sums on
                # partition 0, sqs sums on partition 32) and, afterwards,
                # the r-broadcast G matmul (whose start=True reclaims it).
                gz = gp.tile([128, BC], dt.float32, tag="gz", name="gz")
                mu_row = rows.tile([1, BC], dt.bfloat16, tag="mu_row", name="mu_row")

                for q in range(4):
                    b0 = q * 128
                    zt = zp.tile([128, 12, 128], dt.float32, tag="z", name=f"z_{name}{q}")
                    seen_banks = set()
                    for (i0, ni, j0, nj) in regs:
                        s0 = i0 * Wo + j0
                        n = ni * nj
                        dst = zt[:, s0 : s0 + n, :]
                        # start=True clears has_written for the WHOLE 2KB bank,
                        # so only the first matmul touching a bank may set it.
                        bank = s0 // 4
                        bank_first = bank not in seen_banks
                        seen_banks.add(bank)
                        if first:
                            lhsT = cw1_sb[base : base + 32, :]
                            nc.tensor.matmul(
                                dst, lhsT, rhs_win(0, i0, ni, j0, nj, b0),
                                start=bank_first, stop=True,
                                skip_group_check=not bank_first,
                            )
                        else:
                            t0 = (bi - 2) * 2
                            for t in range(2):
                                lhsT = cw_sb[:, (t0 + t) * 128 : (t0 + t + 1) * 128]
                                nc.tensor.matmul(
                                    dst, lhsT, rhs_win(t, i0, ni, j0, nj, b0),
                                    start=(t == 0 and bank_first), stop=(t == 1),
                                    skip_group_check=not bank_first,
                                )
                    # stats partials for this chunk
                    with nc.allow_low_precision("bf16 groupnorm partial sums"):
                        nc.vector.tensor_reduce(
                            zs[:, b0 : b0 + 128],
                            zt[:, :S, :].rearrange("c s b -> c b s"),
                            axis=mybir.AxisListType.X,
                            op=Alu.add,
                        )
                        sq = sqp.tile([128, 128, 12], dt.bfloat16, tag="sq", name="sq")
                        nc.scalar.activation(
                            sq[:, :, :S],
                            zt[:, :S, :].rearrange("c s b -> c b s"),
                            func=Act.Square,
                        )
                        nc.vector.tensor_reduce(
                            sqs[:, b0 : b0 + 128],
                            sq[:, :, :S],
                            axis=mybir.AxisListType.X,
                            op=Alu.add,
                        )
                    nc.tensor.matmul(
                        gz[0:1, b0 : b0 + 128], ones_col[:], zs[:, b0 : b0 + 128],
                        start=True, stop=True,
                    )
                    nc.tensor.matmul(
                        gz[32:33, b0 : b0 + 128], ones_col[:], sqs[:, b0 : b0 + 128],
                        start=True, stop=True,
                    )
                    nc.scalar.activation(
                        mu_row[:, b0 : b0 + 128], gz[0:1, b0 : b0 + 128],
                        func=Act.Copy, scale=1.0 / CS,
                    )
                    # mean subtraction (K=1 matmul into existing PSUM) + relu evict
                    murow_q = mu_row[:, b0 : b0 + 128]
                    for (i0, ni, j0, nj) in regs:
                        s0 = i0 * Wo + j0
                        n = ni * nj
                        nc.tensor.matmul(
                            zt[:, s0 : s0 + n, :],
                            negones[:],
                            murow_q[:, None, :].to_broadcast((1, n, 128)),
                            start=False, stop=True, skip_group_check=True,
                        )
                    nc.scalar.activation(
                        u_dst[:, :S, b0 : b0 + 128], zt[:, :S, :], func=Act.Relu
                    )

                # per-block row math for r = 1/std
                mu2 = rows.tile([1, BC], dt.float32, tag="mu2", name="mu2")
                nc.scalar.activation(
                    mu2[:], gz[0:1, :], func=Act.Square, scale=1.0 / CS
                )
                ve = rows.tile([1, BC], dt.float32, tag="ve", name="ve")
                nc.vector.scalar_tensor_tensor(
                    ve[:], gz[32:33, :], 1.0 / CS, mu2[:],
                    op0=Alu.mult, op1=Alu.subtract,
                )
                sd = rows.tile([1, BC], dt.float32, tag="sd", name="sd")
                nc.scalar.activation(sd[:], ve[:], func=Act.Sqrt, bias=eps1[:], scale=1.0)
                r0 = rows.tile([1, BC], dt.float32, tag="r0", name="r0")
                nc.vector.reciprocal_approx_fast(out=r0[:], in_=sd[:])
                rb = rows.tile([1, BC], dt.bfloat16, tag="rb", name="rb")
                nc.scalar.activation(rb[:], r0[:], func=Act.Copy)

                # G = r broadcast to 128 partitions (start=True reclaims the
                # stats bank once its rows have been consumed)
                nc.tensor.matmul(gz[:], posones[:], rb[:], start=True, stop=True)
                gsb = gsbp.tile([128, BC], dt.bfloat16, tag="gsb", name="gsb")
                nc.vector.tensor_copy(gsb[:], gz[:])

                # feat = u * r
                nc.gpsimd.tensor_tensor(
                    feat[name][:],
                    u_dst[:, :S, :],
                    gsb[:, None, :].to_broadcast((128, S, BC)),
                    op=Alu.mult,
                )
                if DEBUG_DUMP:
                    if name in dbg_u:
                        nc.sync.dma_start(dbg_u[name][:], u_dst[:])
                    nc.sync.dma_start(dbg_f[name][:], feat[name][:])

                # interleave mtile-0 head matmuls for this block's K-slices
                k0 = sum(S_OF[n] for n, *_ in BLOCKS[:bi])
                for s in range(S):
                    k = k0 + s
                    nc.tensor.matmul(
                        psH0[:],
                        hws01[0][:, k * 128 : (k + 1) * 128],
                        feat[name][:, s, :],
                        start=(k == 0),
                        stop=(k == NK - 1),
                    )

        # ---- heads ----
        hid0 = hidp.tile([128, BC], dt.bfloat16, tag="hid0", name="hid0")
        nc.scalar.activation(
            hid0[:], psH0[:], func=Act.Relu, bias=hb_sb[:, 0:1], scale=1.0
        )
        with (
            tc.tile_pool(name="hp", bufs=2, space="PSUM") as hp,
            tc.tile_pool(name="fp", bufs=1, space="PSUM") as fp,
            tc.tile_pool(name="tp", bufs=2, space="PSUM") as tp,
        ):
            hids = [hid0]
            for mt in range(1, 4):
                if mt == 1:
                    hws = hws01[1]
                else:
                    hws = hwp.tile([128, NK * 128], dt.bfloat16, tag="hws", name=f"hws{mt}")
                    nc.sync.dma_start(hws[:], hw_d[mt])
                psH = hp.tile([128, BC], dt.float32, tag="psH", name="psH")
                k = 0
                for name, _, _, _, _, ho, wo in BLOCKS:
                    for s in range(ho * wo):
                        nc.tensor.matmul(
                            psH[:],
                            hws[:, k * 128 : (k + 1) * 128],
                            feat[name][:, s, :],
                            start=(k == 0),
                            stop=(k == NK - 1),
                        )
                        k += 1
                hid = hidp.tile([128, BC], dt.bfloat16, tag=f"hid{mt}", name=f"hid{mt}")
                nc.scalar.activation(
                    hid[:], psH[:], func=Act.Relu, bias=hb_sb[:, mt : mt + 1], scale=1.0
                )
                hids.append(hid)
            psF = fp.tile([4, BC], dt.float32, tag="psF", name="psF")
            for mt in range(4):
                nc.tensor.matmul(
                    psF[:],
                    fw_sb[:, mt * 4 : (mt + 1) * 4],
                    hids[mt][:],
                    start=(mt == 0),
                    stop=(mt == 3),
                )
            finf = rows.tile([4, BC], dt.float32, tag="finf", name="finf")
            nc.scalar.activation(
                finf[:], psF[:], func=Act.Identity, bias=b2_sb[:, 0:1], scale=1.0
            )
            osb = rows.tile([128, 4, 4], dt.float32, tag="osb", name="osb")
            for qq in range(4):
                psT = tp.tile([128, 4], dt.float32, tag="psT", name="psT")
                nc.tensor.transpose(
                    psT[:], finf[:, qq * 128 : (qq + 1) * 128], ident[0:4, 0:4]
                )
                nc.scalar.copy(osb[:, qq, :], psT[:])
            nc.sync.dma_start(out_d[:].rearrange("(q p) j -> p q j", p=128), osb[:])

    nc.compile()
    return nc


def _prep_weights(inp):
    """Host-side weight preprocessing shared by all cores."""
    f32 = np.float32
    for k in ("b_h1", "b_v1", "b_hh", "b_hv", "b_vh", "b_vv"):
        assert np.allclose(inp[k], 0.0), f"conv bias {k} must be zero"
    for k in ("gb_h1", "gb_v1", "gb_hh", "gb_hv", "gb_vh", "gb_vv"):
        assert np.allclose(inp[k], 0.0), f"groupnorm beta {k} must be zero"
    gammas = {n: np.asarray(inp[f"gw_{n}"], f32) for n in S_OF}
    for n, g in gammas.items():
        assert np.all(g > 0), f"gamma {n} must be positive"

    # first-level conv lhsT: [64, 128] — rows 0:32 h1 taps, 32:64 v1 taps
    w_h1 = np.asarray(inp["w_h1"], f32)
    w_v1 = np.asarray(inp["w_v1"], f32)
    cw1 = np.zeros((64, 128), f32)
    cw1[0:16] = w_h1[:, :, 0, 0].T
    cw1[16:32] = w_h1[:, :, 0, 1].T
    cw1[32:48] = w_v1[:, :, 0, 0].T
    cw1[48:64] = w_v1[:, :, 1, 0].T

    # second-level conv lhsT with parent's gamma folded in
    cw = np.zeros((128, 8 * 128), f32)
    second = [
        ("hh", "w_hh", "h1", "h"),
        ("hv", "w_hv", "h1", "v"),
        ("vh", "w_vh", "v1", "h"),
        ("vv", "w_vv", "v1", "v"),
    ]
    for idx, (name, wk, parent, kind) in enumerate(second):
        w = np.asarray(inp[wk], f32)
        g = gammas[parent]
        for t in range(2):
            tap = w[:, :, 0, t] if kind == "h" else w[:, :, t, 0]
            cw[:, (2 * idx + t) * 128 : (2 * idx + t + 1) * 128] = (tap * g[None, :]).T

    # head weights: W1c = [vw1; aw1] (512, 7424), block gammas folded in,
    # re-tiled per (mtile, block, s)
    W1c = np.concatenate(
        [np.asarray(inp["vw1"], f32), np.asarray(inp["aw1"], f32)], axis=0
    )
    cols = []
    off = 0
    for name, _, _, _, _, ho, wo in BLOCKS:
        S = ho * wo
        Wb = W1c[:, off : off + 128 * S].reshape(512, 128, S)
        Wb = Wb * gammas[name][None, :, None]
        off += 128 * S
        for s in range(S):
            cols.append(Wb[:, :, s])
    K = np.stack(cols, 0)  # (58, 512, 128c)
    hw = np.empty((4, 128, NK * 128), f32)
    for mt in range(4):
        hw[mt] = K[:, mt * 128 : (mt + 1) * 128, :].transpose(2, 0, 1).reshape(128, -1)

    # final layer with dueling algebra folded in
    vw2 = np.asarray(inp["vw2"], f32)  # (1, 256)
    aw2 = np.asarray(inp["aw2"], f32)  # (4, 256)
    W2c = np.zeros((4, 512), f32)
    W2c[:, 0:256] = vw2[0][None, :]
    W2c[:, 256:512] = aw2 - aw2.mean(axis=0, keepdims=True)
    W2cT = W2c.T  # (512, 4)
    fw = np.zeros((128, 16), f32)
    for kt in range(4):
        fw[:, kt * 4 : (kt + 1) * 4] = W2cT[kt * 128 : (kt + 1) * 128, :]
    b2 = (
        np.asarray(inp["vb2"], f32)[0]
        + np.asarray(inp["ab2"], f32)
        - np.asarray(inp["ab2"], f32).mean()
    ).reshape(4, 1)

    hb = np.concatenate(
        [np.asarray(inp["vb1"], f32), np.asarray(inp["ab1"], f32)]
    ).reshape(4, 128).T.copy()  # [128, 4], column mt

    return {
        "cw1": cw1.astype(BF16),
        "cw": cw.astype(BF16),
        "hw": hw.astype(BF16),
        "fw": fw.astype(BF16),
        "hb": hb.astype(np.float32),
        "b2": b2.astype(np.float32),
    }


def _prep_x(xs):
    """Per-core input prep: [64, 16, n] tap-stacked bf16 (h taps 0:32, v 32:64)."""
    f32 = np.float32
    n = xs.shape[0]
    xt = np.zeros((n, 64, 4, 4), f32)
    xt[:, 0:16] = xs
    xt[:, 16:32, :, 0:3] = xs[:, :, :, 1:4]
    xt[:, 32:48] = xs
    xt[:, 48:64, 0:3, :] = xs[:, :, 1:4, :]
    xt = xt.transpose(1, 2, 3, 0).reshape(64, 16, n)
    return xt.astype(BF16)


def _get_nc():
    if "nc" not in _cache:
        _cache["nc"] = _build()
    return _cache["nc"]


def _core_in_map(inputs, c, _wcache={}):
    key = id(inputs)
    if _wcache.get("key") != key:
        _wcache["key"] = key
        _wcache["w"] = _prep_weights(inputs)
        _wcache["x"] = np.asarray(inputs["x"], np.float32)
    x = _wcache["x"]
    m = dict(_wcache["w"])
    m["xt"] = _prep_x(x[c * BC : (c + 1) * BC])
    return m


def _unpack_out(out):
    return np.asarray(out, np.float32)


def kernel(**inputs) -> np.ndarray:
    from concourse.bass_utils import run_bass_kernel_spmd

    nc = _get_nc()
    in_maps = [_core_in_map(inputs, c) for c in range(NCORES)]
    res = run_bass_kernel_spmd(nc, in_maps, core_ids=list(range(NCORES)))
    out = np.concatenate([_unpack_out(r["out"]) for r in res.results], axis=0)
    return out.astype(np.float32)


# revision 21
# speedup vs baseline: 1.1918x; 1.1918x over previous
"""Trainium2 Bass kernel for nn_CNNQNetwork (dueling CNN Q-network).

Sharding: pure data parallel — batch 4096 split as 512 samples on each of the
8 NeuronCores; all weights replicated.

Per-core layout: activations in SBUF as [channel(partition), spatial, batch].
Per (block, q=128-sample chunk): conv taps accumulate into a PSUM tile
[128, S, 128] (3 banks); matmuls are split at 2KB PSUM bank boundaries
(rectangular (i,j) sub-windows). GroupNorm(1 group):
  - zs[c,b] = sum_s z via DVE reduce (PSUM f32)
  - sq = z^2 via ScalarE -> [c,b,s] bf16, sqs[c,b] via dense DVE reduce
  - cross-channel sums via ones-matmul -> [1,512] rows in one PSUM bank
  - row math: mu (ScalarE copy*1/CS), mu2 (ScalarE square*1/CS),
    ve (DVE STT), sd=sqrt(ve+eps) (ScalarE), r=1/sd (DVE recip_approx_fast)
  - mean subtraction folded into the conv PSUM as a K=1 matmul (lhsT=-1,
    rhs=mu row broadcast); u = relu(z - mu) (ScalarE evict)
The head runs directly on the UNNORMALIZED u: for each (mtile, block) the
W1^T u partial accumulates in PSUM, then hidden_acc += partial * r
(r broadcast to 128 partitions via GpSimd partition_broadcast; the multiply
is a GpSimd tensor_tensor, the accumulate a DVE add). This removes the
feature-scaling pass entirely, and head matmuls interleave into conv-phase
PE gaps (emission is software-pipelined one block behind the convs).
Per-channel gammas are folded into consuming conv weights and the head W1
host-side (GroupNorm scale invariance makes unnormalized u safe as conv
input). The dueling-head algebra is folded into the second linear layer
host-side. All four head-weight mtiles stream from HBM during the convs.
"""

import numpy as np
import ml_dtypes

BF16 = ml_dtypes.bfloat16
B_TOTAL = 4096
NCORES = 8
BC = B_TOTAL // NCORES  # 512 samples per core
D = 128
EPS = 1e-5

# blocks: (name, src, kind, Hi, Wi, Ho, Wo)   kind 'h' = (1,2) kernel, 'v' = (2,1)
BLOCKS = [
    ("h1", "x", "h", 4, 4, 4, 3),
    ("v1", "x", "v", 4, 4, 3, 4),
    ("hh", "h1", "h", 4, 3, 4, 2),
    ("hv", "h1", "v", 4, 3, 3, 3),
    ("vh", "v1", "h", 3, 4, 3, 3),
    ("vv", "v1", "v", 3, 4, 2, 4),
]
S_OF = {n: ho * wo for (n, _, _, _, _, ho, wo) in BLOCKS}
NK = sum(S_OF.values())  # 58 K-slices of 128 for the head matmul

_cache = {}
DEBUG_DUMP = False


def _regions(Ho, Wo):
    """Rectangular (i0, ni, j0, nj) output sub-windows whose flattened
    (s, b128) PSUM columns stay within one 2KB bank (4 s-positions)."""
    regs = []
    if Wo == 4:
        for i in range(Ho):
            regs.append((i, 1, 0, 4))
    elif Wo == 2:
        for i in range(0, Ho, 2):
            regs.append((i, 2, 0, 2))
    else:  # Wo == 3
        for i in range(Ho):
            j = 0
            while j < Wo:
                s = i * Wo + j
                jn = min(Wo - j, 4 - (s % 4))
                regs.append((i, 1, j, jn))
                j += jn
    return regs


def _build():
    """Build the Bass program once. Returns nc."""
    import concourse.bass as bass
    import concourse.tile as tile
    import concourse.mybir as mybir
    from concourse import bacc
    from concourse.masks import make_identity
    from contextlib import ExitStack

    dt = mybir.dt
    Alu = mybir.AluOpType
    Act = mybir.ActivationFunctionType

    nc = bacc.Bacc(
        "TRN2",
        target_bir_lowering=False,
        debug=False,
        enable_asserts=False,
        num_devices=NCORES,
    )

    # ---- DRAM I/O ----
    xt_d = nc.dram_tensor("xt", [64, 16, BC], dt.bfloat16, kind="ExternalInput")
    cw1_d = nc.dram_tensor("cw1", [64, 128], dt.bfloat16, kind="ExternalInput")
    cw_d = nc.dram_tensor("cw", [128, 8 * 128], dt.bfloat16, kind="ExternalInput")
    hw_d = nc.dram_tensor("hw", [4, 128, NK * 128], dt.bfloat16, kind="ExternalInput")
    fw_d = nc.dram_tensor("fw", [128, 16], dt.bfloat16, kind="ExternalInput")
    hb_d = nc.dram_tensor("hb", [128, 4], dt.float32, kind="ExternalInput")
    b2_d = nc.dram_tensor("b2", [4, 1], dt.float32, kind="ExternalInput")
    out_d = nc.dram_tensor("out", [BC, 4], dt.float32, kind="ExternalOutput")
    if DEBUG_DUMP:
        dbg_u = {
            n: nc.dram_tensor(f"dbg_u_{n}", [128, S_OF[n], BC], dt.bfloat16, kind="ExternalOutput")
            for n in S_OF
        }
        dbg_acc = nc.dram_tensor("dbg_acc", [4, 128, BC], dt.float32, kind="ExternalOutput")

    with tile.TileContext(nc) as tc, ExitStack() as ctx:
        singles = ctx.enter_context(tc.tile_pool(name="singles", bufs=1))
        rows = ctx.enter_context(tc.tile_pool(name="rows", bufs=2))
        sqp = ctx.enter_context(tc.tile_pool(name="sqp", bufs=3))
        statp = ctx.enter_context(tc.tile_pool(name="statp", bufs=2))
        gsbp = ctx.enter_context(tc.tile_pool(name="gsbp", bufs=2))
        tmpp = ctx.enter_context(tc.tile_pool(name="tmpp", bufs=2))

        # input + conv weights first in the DMA queues: convs start early
        xt_sb = singles.tile([64, 16, BC], dt.bfloat16, tag="xt", name="xt")
        cw1_sb = singles.tile([64, 128], dt.bfloat16, tag="cw1", name="cw1")
        cw_sb = singles.tile([128, 8 * 128], dt.bfloat16, tag="cw", name="cw")
        for q in range(4):
            b0 = q * 128
            nc.sync.dma_start(xt_sb[:, :, b0 : b0 + 128], xt_d[:, :, b0 : b0 + 128])
        nc.sync.dma_start(cw1_sb[:], cw1_d[:])
        nc.sync.dma_start(cw_sb[:], cw_d[:])

        fw_sb = singles.tile([128, 16], dt.bfloat16, tag="fw", name="fw")
        hb_sb = singles.tile([128, 4], dt.float32, tag="hb", name="hb")
        b2_sb = singles.tile([4, 1], dt.float32, tag="b2", name="b2")
        ident = singles.tile([128, 128], dt.float32, tag="ident", name="ident")
        ones_col = singles.tile([128, 1], dt.bfloat16, tag="ones_col", name="ones_col")
        negones = singles.tile([1, 128], dt.bfloat16, tag="negones", name="negones")
        eps1 = singles.tile([1, 1], dt.float32, tag="eps1", name="eps1")
        nc.vector.memset(eps1[:], EPS)
        nc.vector.memset(ones_col[:], 1.0)
        nc.vector.memset(negones[:], -1.0)
        nc.sync.dma_start(fw_sb[:], fw_d[:])
        nc.sync.dma_start(hb_sb[:], hb_d[:])
        nc.sync.dma_start(b2_sb[:], b2_d[:])
        make_identity(nc, ident[:])

        # all four head-weight mtiles stream during the conv phase
        hws = []
        for mt in range(4):
            h = singles.tile([128, NK * 128], dt.bfloat16, tag=f"hws{mt}", name=f"hws{mt}")
            nc.sync.dma_start(h[:], hw_d[mt])
            hws.append(h)

        # unnormalized post-relu activations, persistent per block
        u_of = {}
        for name, _, _, _, _, ho, wo in BLOCKS:
            s = 12 if name in ("h1", "v1") else ho * wo
            u_of[name] = singles.tile([128, s, BC], dt.bfloat16, tag=f"u_{name}", name=f"u_{name}")
        # per-mtile hidden accumulators (f32, SBUF)
        acc = [
            singles.tile([128, BC], dt.float32, tag=f"acc{mt}", name=f"acc{mt}")
            for mt in range(4)
        ]
        gsb_of = {}

        def emit_block(bi):
            name, src, kind, Hi, Wi, Ho, Wo = BLOCKS[bi]
            S = Ho * Wo
            CS = 128 * S
            first = src == "x"
            regs = _regions(Ho, Wo)

            if first:
                base = 0 if kind == "h" else 32
                sview = xt_sb[base : base + 32].rearrange("c (i j) b -> c i j b", i=Hi)
            else:
                sview = u_of[src][:, : Hi * Wi, :].rearrange(
                    "c (i j) b -> c i j b", i=Hi
                )

            def rhs_win(t, i0, ni, j0, nj, b0):
                if kind == "h":
                    return sview[:, i0 : i0 + ni, j0 + t : j0 + t + nj, b0 : b0 + 128]
                else:
                    return sview[:, i0 + t : i0 + t + ni, j0 : j0 + nj, b0 : b0 + 128]

            u_dst = u_of[name]
            zs = statp.tile([128, BC], dt.bfloat16, tag="zs", name="zs")
            sqs = statp.tile([128, BC], dt.bfloat16, tag="sqs", name="sqs")
            # stats rows: zs sums on partition 0, sqs sums on partition 32
            psZQ = sp.tile([64, BC], dt.float32, tag="zq", name="psZQ")
            mu_row = rows.tile([1, BC], dt.bfloat16, tag="mu_row", name="mu_row")

            for q in range(4):
                b0 = q * 128
                zt = zp.tile([128, 12, 128], dt.float32, tag="z", name=f"z_{name}{q}")
                seen_banks = set()
                for (i0, ni, j0, nj) in regs:
                    s0 = i0 * Wo + j0
                    n = ni * nj
                    dst = zt[:, s0 : s0 + n, :]
                    # start=True clears has_written for the WHOLE 2KB bank,
                    # so only the first matmul touching a bank may set it.
                    bank = s0 // 4
                    bank_first = bank not in seen_banks
                    seen_banks.add(bank)
                    if first:
                        lhsT = cw1_sb[base : base + 32, :]
                        nc.tensor.matmul(
                            dst, lhsT, rhs_win(0, i0, ni, j0, nj, b0),
                            start=bank_first, stop=True,
                            skip_group_check=not bank_first,
                        )
                    else:
                        t0 = (bi - 2) * 2
                        for t in range(2):
                            lhsT = cw_sb[:, (t0 + t) * 128 : (t0 + t + 1) * 128]
                            nc.tensor.matmul(
                                dst, lhsT, rhs_win(t, i0, ni, j0, nj, b0),
                                start=(t == 0 and bank_first), stop=(t == 1),
                                skip_group_check=not bank_first,
                            )
                # stats partials for this chunk
                with nc.allow_low_precision("bf16 groupnorm partial sums"):
                    nc.vector.tensor_reduce(
                        zs[:, b0 : b0 + 128],
                        zt[:, :S, :].rearrange("c s b -> c b s"),
                        axis=mybir.AxisListType.X,
                        op=Alu.add,
                    )
                    sq = sqp.tile([128, 128, 12], dt.bfloat16, tag="sq", name="sq")
                    nc.scalar.activation(
                        sq[:, :, :S],
                        zt[:, :S, :].rearrange("c s b -> c b s"),
                        func=Act.Square,
                    )
                    nc.vector.tensor_reduce(
                        sqs[:, b0 : b0 + 128],
                        sq[:, :, :S],
                        axis=mybir.AxisListType.X,
                        op=Alu.add,
                    )
                nc.tensor.matmul(
                    psZQ[0:1, b0 : b0 + 128], ones_col[:], zs[:, b0 : b0 + 128],
                    start=True, stop=True,
                )
                nc.tensor.matmul(
                    psZQ[32:33, b0 : b0 + 128], ones_col[:], sqs[:, b0 : b0 + 128],
                    start=True, stop=True,
                )
                nc.scalar.activation(
                    mu_row[:, b0 : b0 + 128], psZQ[0:1, b0 : b0 + 128],
                    func=Act.Copy, scale=1.0 / CS,
                )
                # mean subtraction (K=1 matmul into existing PSUM) + relu evict
                murow_q = mu_row[:, b0 : b0 + 128]
                for (i0, ni, j0, nj) in regs:
                    s0 = i0 * Wo + j0
                    n = ni * nj
                    nc.tensor.matmul(
                        zt[:, s0 : s0 + n, :],
                        negones[:],
                        murow_q[:, None, :].to_broadcast((1, n, 128)),
                        start=False, stop=True, skip_group_check=True,
                    )
                nc.scalar.activation(
                    u_dst[:, :S, b0 : b0 + 128], zt[:, :S, :], func=Act.Relu
                )

            # per-block row math for r = 1/std
            mu2 = rows.tile([1, BC], dt.float32, tag="mu2", name="mu2")
            nc.scalar.activation(mu2[:], psZQ[0:1, :], func=Act.Square, scale=1.0 / CS)
            ve = rows.tile([1, BC], dt.float32, tag="ve", name="ve")
            nc.vector.scalar_tensor_tensor(
                ve[:], psZQ[32:33, :], 1.0 / CS, mu2[:],
                op0=Alu.mult, op1=Alu.subtract,
            )
            sd = rows.tile([1, BC], dt.float32, tag="sd", name="sd")
            nc.scalar.activation(sd[:], ve[:], func=Act.Sqrt, bias=eps1[:], scale=1.0)
            r0 = rows.tile([1, BC], dt.float32, tag="r0", name="r0")
            nc.vector.reciprocal_approx_fast(out=r0[:], in_=sd[:])
            rb = rows.tile([1, BC], dt.bfloat16, tag="rb", name="rb")
            nc.scalar.activation(rb[:], r0[:], func=Act.Copy)
            # r broadcast to all 128 partitions for the head combine
            gsb = gsbp.tile([128, BC], dt.bfloat16, tag="gsb", name=f"gsb_{name}")
            nc.gpsimd.partition_broadcast(gsb[:], rb[:])
            gsb_of[name] = gsb
            if DEBUG_DUMP:
                nc.sync.dma_start(dbg_u[name][:], u_dst[:, :S, :])

        def emit_heads(bi):
            name = BLOCKS[bi][0]
            S = S_OF[name]
            k0 = sum(S_OF[n] for n, *_ in BLOCKS[:bi])
            gsb = gsb_of[name]
            for mt in range(4):
                psHB = hbp.tile([128, BC], dt.float32, tag="psHB", name=f"psHB_{name}{mt}")
                for s in range(S):
                    k = k0 + s
                    nc.tensor.matmul(
                        psHB[:],
                        hws[mt][:, k * 128 : (k + 1) * 128],
                        u_of[name][:, s, :],
                        start=(s == 0),
                        stop=(s == S - 1),
                    )
                if bi == 0:
                    nc.vector.tensor_tensor(acc[mt][:], psHB[:], gsb[:], op=Alu.mult)
                else:
                    tmp = tmpp.tile([128, BC], dt.bfloat16, tag="tmp", name="tmp")
                    nc.vector.tensor_tensor(tmp[:], psHB[:], gsb[:], op=Alu.mult)
                    nc.gpsimd.tensor_tensor(acc[mt][:], acc[mt][:], tmp[:], op=Alu.add)

        with (
            tc.tile_pool(name="zp", bufs=2, space="PSUM") as zp,
            tc.tile_pool(name="sp", bufs=1, space="PSUM") as sp,
            tc.tile_pool(name="hbp", bufs=1, space="PSUM") as hbp,
        ):
            for bi in range(6):
                emit_block(bi)
                if bi >= 1:
                    emit_heads(bi - 1)
            emit_heads(5)

        if DEBUG_DUMP:
            for mt in range(4):
                nc.sync.dma_start(dbg_acc[mt], acc[mt][:])

        # ---- final layer ----
        with (
            tc.tile_pool(name="fp", bufs=1, space="PSUM") as fp,
            tc.tile_pool(name="tp", bufs=2, space="PSUM") as tp,
        ):
            hids = []
            for mt in range(4):
                hid = singles.tile([128, BC], dt.bfloat16, tag=f"hid{mt}", name=f"hid{mt}")
                nc.scalar.activation(
                    hid[:], acc[mt][:], func=Act.Relu, bias=hb_sb[:, mt : mt + 1], scale=1.0
                )
                hids.append(hid)
            psF = fp.tile([4, BC], dt.float32, tag="psF", name="psF")
            for mt in range(4):
                nc.tensor.matmul(
                    psF[:],
                    fw_sb[:, mt * 4 : (mt + 1) * 4],
                    hids[mt][:],
                    start=(mt == 0),
                    stop=(mt == 3),
                )
            finf = rows.tile([4, BC], dt.float32, tag="finf", name="finf")
            nc.scalar.activation(
                finf[:], psF[:], func=Act.Identity, bias=b2_sb[:, 0:1], scale=1.0
            )
            osb = rows.tile([128, 4, 4], dt.float32, tag="osb", name="osb")
            for qq in range(4):
                psT = tp.tile([128, 4], dt.float32, tag="psT", name="psT")
                nc.tensor.transpose(
                    psT[:], finf[:, qq * 128 : (qq + 1) * 128], ident[0:4, 0:4]
                )
                nc.scalar.copy(osb[:, qq, :], psT[:])
            nc.sync.dma_start(out_d[:].rearrange("(q p) j -> p q j", p=128), osb[:])

    nc.compile()
    return nc


def _prep_weights(inp):
    """Host-side weight preprocessing shared by all cores."""
    f32 = np.float32
    for k in ("b_h1", "b_v1", "b_hh", "b_hv", "b_vh", "b_vv"):
        assert np.allclose(inp[k], 0.0), f"conv bias {k} must be zero"
    for k in ("gb_h1", "gb_v1", "gb_hh", "gb_hv", "gb_vh", "gb_vv"):
        assert np.allclose(inp[k], 0.0), f"groupnorm beta {k} must be zero"
    gammas = {n: np.asarray(inp[f"gw_{n}"], f32) for n in S_OF}
    for n, g in gammas.items():
        assert np.all(g > 0), f"gamma {n} must be positive"

    # first-level conv lhsT: [64, 128] — rows 0:32 h1 taps, 32:64 v1 taps
    w_h1 = np.asarray(inp["w_h1"], f32)
    w_v1 = np.asarray(inp["w_v1"], f32)
    cw1 = np.zeros((64, 128), f32)
    cw1[0:16] = w_h1[:, :, 0, 0].T
    cw1[16:32] = w_h1[:, :, 0, 1].T
    cw1[32:48] = w_v1[:, :, 0, 0].T
    cw1[48:64] = w_v1[:, :, 1, 0].T

    # second-level conv lhsT with parent's gamma folded in
    cw = np.zeros((128, 8 * 128), f32)
    second = [
        ("hh", "w_hh", "h1", "h"),
        ("hv", "w_hv", "h1", "v"),
        ("vh", "w_vh", "v1", "h"),
        ("vv", "w_vv", "v1", "v"),
    ]
    for idx, (name, wk, parent, kind) in enumerate(second):
        w = np.asarray(inp[wk], f32)
        g = gammas[parent]
        for t in range(2):
            tap = w[:, :, 0, t] if kind == "h" else w[:, :, t, 0]
            cw[:, (2 * idx + t) * 128 : (2 * idx + t + 1) * 128] = (tap * g[None, :]).T

    # head weights: W1c = [vw1; aw1] (512, 7424), block gammas folded in,
    # re-tiled per (mtile, block, s)
    W1c = np.concatenate(
        [np.asarray(inp["vw1"], f32), np.asarray(inp["aw1"], f32)], axis=0
    )
    cols = []
    off = 0
    for name, _, _, _, _, ho, wo in BLOCKS:
        S = ho * wo
        Wb = W1c[:, off : off + 128 * S].reshape(512, 128, S)
        Wb = Wb * gammas[name][None, :, None]
        off += 128 * S
        for s in range(S):
            cols.append(Wb[:, :, s])
    K = np.stack(cols, 0)  # (58, 512, 128c)
    hw = np.empty((4, 128, NK * 128), f32)
    for mt in range(4):
        hw[mt] = K[:, mt * 128 : (mt + 1) * 128, :].transpose(2, 0, 1).reshape(128, -1)

    # final layer with dueling algebra folded in
    vw2 = np.asarray(inp["vw2"], f32)  # (1, 256)
    aw2 = np.asarray(inp["aw2"], f32)  # (4, 256)
    W2c = np.zeros((4, 512), f32)
    W2c[:, 0:256] = vw2[0][None, :]
    W2c[:, 256:512] = aw2 - aw2.mean(axis=0, keepdims=True)
    W2cT = W2c.T  # (512, 4)
    fw = np.zeros((128, 16), f32)
    for kt in range(4):
        fw[:, kt * 4 : (kt + 1) * 4] = W2cT[kt * 128 : (kt + 1) * 128, :]
    b2 = (
        np.asarray(inp["vb2"], f32)[0]
        + np.asarray(inp["ab2"], f32)
        - np.asarray(inp["ab2"], f32).mean()
    ).reshape(4, 1)

    hb = np.concatenate(
        [np.asarray(inp["vb1"], f32), np.asarray(inp["ab1"], f32)]
    ).reshape(4, 128).T.copy()  # [128, 4], column mt

    return {
        "cw1": cw1.astype(BF16),
        "cw": cw.astype(BF16),
        "hw": hw.astype(BF16),
        "fw": fw.astype(BF16),
        "hb": hb.astype(np.float32),
        "b2": b2.astype(np.float32),
    }


def _prep_x(xs):
    """Per-core input prep: [64, 16, n] tap-stacked bf16 (h taps 0:32, v 32:64)."""
    f32 = np.float32
    n = xs.shape[0]
    xt = np.zeros((n, 64, 4, 4), f32)
    xt[:, 0:16] = xs
    xt[:, 16:32, :, 0:3] = xs[:, :, :, 1:4]
    xt[:, 32:48] = xs
    xt[:, 48:64, 0:3, :] = xs[:, :, 1:4, :]
    xt = xt.transpose(1, 2, 3, 0).reshape(64, 16, n)
    return xt.astype(BF16)


def _get_nc():
    if "nc" not in _cache:
        _cache["nc"] = _build()
    return _cache["nc"]


def _core_in_map(inputs, c, _wcache={}):
    key = id(inputs)
    if _wcache.get("key") != key:
        _wcache["key"] = key
        _wcache["w"] = _prep_weights(inputs)
        _wcache["x"] = np.asarray(inputs["x"], np.float32)
    x = _wcache["x"]
    m = dict(_wcache["w"])
    m["xt"] = _prep_x(x[c * BC : (c + 1) * BC])
    return m


def _unpack_out(out):
    return np.asarray(out, np.float32)


def kernel(**inputs) -> np.ndarray:
    from concourse.bass_utils import run_bass_kernel_spmd

    nc = _get_nc()
    in_maps = [_core_in_map(inputs, c) for c in range(NCORES)]
    res = run_bass_kernel_spmd(nc, in_maps, core_ids=list(range(NCORES)))
    out = np.concatenate([_unpack_out(r["out"]) for r in res.results], axis=0)
    return out.astype(np.float32)
